# revision 19
# baseline (speedup 1.0000x reference)
"""Trainium2 Bass kernel for CustomHyperSemanticMessagePassing.

Hypergraph multi-head attention message passing, N=4096 nodes, E=4096 edges,
DEG=CARD=8, D=256, H=8 heads. Sharding: data-parallel over nodes (512/core).

Per core:
  phase T: build K|V row table (fp16, (e,h)-interleaved columns) and EK table
           with PE matmuls from transposed inputs; q for own nodes.
  phase Q: gather per-node EK rows for all 8 rounds, compute qek[n,r,h].
  phase A: per (128-node tile, round-pair): one batched indirect DMA gathers
           16 K|V rows per node; scores via fp16 multiply + halving-tree
           reductions (DVE 2x mode), exp on Act, weighted V-sum via fp16
           multiply + tree (split across DVE/Pool).
  phase O: softmax-normalize, transpose, out-proj on PE, relu, store.

All elementwise tensors keep a 2-byte dtype with the innermost dim packed so
the DVE runs in its 2x performance mode; TensorReduce (no fast mode) is
replaced by log2 trees of TensorTensor adds.
"""
import numpy as np

import bass_rust
import orjson
import concourse.bass as bass
import concourse.tile as tile
import concourse.bass_utils as bass_utils
import concourse.bass2jax as bass2jax
from concourse import mybir
from concourse.masks import make_identity

F32 = mybir.dt.float32
F16 = mybir.dt.bfloat16
F8 = mybir.dt.float8e4
I32 = mybir.dt.int32

N, E, D, EDGE_DIM = 4096, 4096, 256, 64
H, DH, DEG, CARD = 8, 32, 8, 8
L = DEG * CARD
NCORES = 8
NSH = N // NCORES          # nodes per core
NT = NSH // 128            # 128-node tiles per core
RP = 4                     # rounds per gather unit
NU = DEG // RP             # units per tile
UP = RP * CARD             # pairs per unit

ADD = mybir.AluOpType.add
MULT = mybir.AluOpType.mult


# ---------------------------------------------------------------------------
# walrus workaround: this build accepts only one sync-wait per instruction;
# split extras into injected single-wait NoOps at the BIR-JSON level.
_ORIG_COMPILE = bass_utils.compile_bir_kernel
_ctr = [0]


def _split_multiwaits(bir_json: bytes) -> bytes:
    bir = orjson.loads(bir_json)
    changed = False
    for f in bir.get("functions", []):
        for blk in f.get("blocks", []):
            out = []
            for ins in blk.get("instructions", []):
                si = ins.get("sync_info")
                waits = (si or {}).get("on_wait") or []
                if len(waits) > 1 and ins.get("engine") not in (None, "Unassigned"):
                    changed = True
                    for w in waits[:-1]:
                        _ctr[0] += 1
                        out.append({
                            "debug": ins.get("debug"),
                            "engine": ins["engine"],
                            "ins": [], "outs": [],
                            "name": f"WSPLIT-{_ctr[0]}",
                            "opcode": "NoOp",
                            "sync_info": {"on_wait": [w], "on_update": []},
                        })
                    si["on_wait"] = waits[-1:]
                out.append(ins)
            blk["instructions"] = out
    return orjson.dumps(bir) if changed else bir_json


def _patched_compile(bir_json, tmpdir, neff_name="file.neff"):
    return _ORIG_COMPILE(_split_multiwaits(bytes(bir_json)), tmpdir,
                         neff_name=neff_name)


def _install_patch():
    bass_utils.compile_bir_kernel = _patched_compile
    bass2jax.compile_bir_kernel = _patched_compile


_install_patch()


# ---------------------------------------------------------------------------
def build_nc():
    nc = bass.Bass(num_devices=NCORES)
    # replicated inputs (fp16, transposed)
    xT = nc.declare_dram_parameter("xT", [D, N], F16, isOutput=False)
    eaT = nc.declare_dram_parameter("eaT", [EDGE_DIM, E], F16, isOutput=False)
    wk = nc.declare_dram_parameter("wk", [D, D], F16, isOutput=False)
    wv = nc.declare_dram_parameter("wv", [D, D], F16, isOutput=False)
    wq = nc.declare_dram_parameter("wq", [D, D], F16, isOutput=False)
    wek = nc.declare_dram_parameter("wek", [EDGE_DIM, D], F16, isOutput=False)
    owT = nc.declare_dram_parameter("owT", [D, D], F16, isOutput=False)
    # per-core inputs
    xT_own = nc.declare_dram_parameter("xT_own", [D, NSH], F16, isOutput=False)
    pu = nc.declare_dram_parameter("pu", [NSH, L], I32, isOutput=False)
    pe_ = nc.declare_dram_parameter("pe", [NSH, DEG], I32, isOutput=False)
    # output
    out = nc.declare_dram_parameter("out", [NSH, D], F32, isOutput=True)
    # internal tables
    k_tab = nc.dram_tensor("k_tab", [N, D], F16)
    v_tab = nc.dram_tensor("v_tab", [N, D], F16)
    ek_tab = nc.dram_tensor("ek_tab", [E, D], F16)

    import contextlib
    with tile.TileContext(nc) as tc, \
         nc.allow_low_precision(reason="fp16 trees validated vs reference"), \
         contextlib.ExitStack() as es:
        wp = es.enter_context(tc.tile_pool(name="wpool", bufs=1))
        xp = es.enter_context(tc.tile_pool(name="xpool", bufs=2))
        xsp = es.enter_context(tc.tile_pool(name="xspool", bufs=1))
        tp = es.enter_context(tc.tile_pool(name="tpool", bufs=2))
        qp = es.enter_context(tc.tile_pool(name="qpool", bufs=1))
        ip = es.enter_context(tc.tile_pool(name="ipool", bufs=1))
        gkp = es.enter_context(tc.tile_pool(name="gkpool", bufs=3))
        gvp = es.enter_context(tc.tile_pool(name="gvpool", bufs=3))
        sdp = es.enter_context(tc.tile_pool(name="sdpool", bufs=2))
        wep = es.enter_context(tc.tile_pool(name="wepool", bufs=4))
        cp = es.enter_context(tc.tile_pool(name="cpool", bufs=1))
        zp = es.enter_context(tc.tile_pool(name="zpool", bufs=1))
        op_ = es.enter_context(tc.tile_pool(name="opool", bufs=2))
        psA = es.enter_context(tc.tile_pool(name="psA", bufs=2, space="PSUM"))
        psB = es.enter_context(tc.tile_pool(name="psB", bufs=2, space="PSUM"))
        psE = es.enter_context(tc.tile_pool(name="psE", bufs=1, space="PSUM"))
        psC = es.enter_context(tc.tile_pool(name="psC", bufs=1, space="PSUM"))

        # ---- resident weights / inputs ----
        wk_t = wp.tile([128, 2, D], F16)
        nc.sync.dma_start(out=wk_t[:], in_=wk[:].rearrange("(c k) o -> k c o", c=2))
        wv_t = wp.tile([128, 2, D], F16)
        nc.sync.dma_start(out=wv_t[:], in_=wv[:].rearrange("(c k) o -> k c o", c=2))
        wq_t = wp.tile([128, 2, D], F16)
        nc.sync.dma_start(out=wq_t[:], in_=wq[:].rearrange("(c k) o -> k c o", c=2))
        wek_t = wp.tile([EDGE_DIM, D], F16)
        nc.sync.dma_start(out=wek_t[:], in_=wek[:])
        owT_t = wp.tile([128, 2, D], F16)
        nc.sync.dma_start(out=owT_t[:], in_=owT[:].rearrange("(c k) o -> k c o", c=2))
        ident = wp.tile([128, 128], F32)
        make_identity(nc, ident[:])

        ea_sb = xsp.tile([EDGE_DIM, E], F16, tag="easb")
        nc.sync.dma_start(out=ea_sb[:], in_=eaT[:])
        xo_sb = xsp.tile([128, 2, NSH], F16, tag="xosb")
        nc.sync.dma_start(out=xo_sb[:], in_=xT_own[:].rearrange("(c k) n -> k c n", c=2))

        # ---- phase T: EK table ----
        for m in range(E // 128):
            mb_, mi = divmod(m, 4)
            if mi % 2 == 0:
                pek = psE.tile([128, 2, D], F32, space="PSUM", tag="pek2")
            nc.tensor.matmul(out=pek[:, mi % 2, :], lhsT=ea_sb[:, bass.ts(m, 128)],
                             rhs=wek_t[:], start=True, stop=True)
            if mi == 0:
                ek_st = tp.tile([128, 4, D], F16, tag="ekst")
            if mi % 2 == 1:
                nc.scalar.copy(out=ek_st[:, mi - 1:mi + 1, :], in_=pek[:])
            if mi == 3:
                nc.scalar.dma_start(
                    out=ek_tab[bass.ts(mb_, 512), :].rearrange(
                        "(q p) d -> p q d", p=128), in_=ek_st[:])

        # ---- phase T: q for own nodes (resident) ----
        q_tiles = []
        for t in range(NT):
            pq = psB.tile([128, D], F32, space="PSUM", tag="p256")
            nc.tensor.matmul(out=pq[:], lhsT=xo_sb[:, 0, bass.ts(t, 128)],
                             rhs=wq_t[:, 0, :], start=True, stop=False)
            nc.tensor.matmul(out=pq[:], lhsT=xo_sb[:, 1, bass.ts(t, 128)],
                             rhs=wq_t[:, 1, :], start=False, stop=True)
            q_t = qp.tile([128, D], F16, tag=f"q{t}")
            nc.scalar.copy(out=q_t[:], in_=pq[:])
            q_tiles.append(q_t)

        # ---- phase Q: per-tile round-edge EK gather + qek ----
        pu_tiles, qek_tiles = [], []
        for t in range(NT):
            pu_t = ip.tile([128, L], I32, tag=f"put{t}")
            nc.gpsimd.dma_start(out=pu_t[:], in_=pu[bass.ts(t, 128), :])
            pe_t = ip.tile([128, DEG], I32, tag=f"pet{t}")
            nc.gpsimd.dma_start(out=pe_t[:], in_=pe_[bass.ts(t, 128), :])
            pu_tiles.append(pu_t)

            ekg = xp.tile([128, DEG, D], F16, tag="ekg")
            nc.gpsimd.indirect_dma_start(
                out=ekg[:], out_offset=None, in_=ek_tab[:],
                in_offset=bass.IndirectOffsetOnAxis(ap=pe_t[:], axis=0))
            nc.vector.tensor_tensor(
                out=ekg[:], in0=ekg[:],
                in1=q_tiles[t][:].unsqueeze(1).to_broadcast([128, DEG, D]),
                op=MULT)
            pv = ekg[:].rearrange("p r (e h) -> p r e h", h=H)
            nc.vector.tensor_tensor(out=pv[:, :, 0:16, :], in0=pv[:, :, 0:16, :],
                                    in1=pv[:, :, 16:32, :], op=ADD)
            nc.vector.tensor_tensor(out=pv[:, :, 0:8, :], in0=pv[:, :, 0:8, :],
                                    in1=pv[:, :, 8:16, :], op=ADD)
            nc.vector.tensor_tensor(out=pv[:, :, 0:4, :], in0=pv[:, :, 0:4, :],
                                    in1=pv[:, :, 4:8, :], op=ADD)
            nc.vector.tensor_tensor(out=pv[:, :, 0:2, :], in0=pv[:, :, 0:2, :],
                                    in1=pv[:, :, 2:4, :], op=ADD)
            qek_t = qp.tile([128, DEG, H], F16, tag=f"qek{t}")
            nc.vector.tensor_tensor(out=qek_t[:], in0=pv[:, :, 0, :],
                                    in1=pv[:, :, 1, :], op=ADD)
            qek_tiles.append(qek_t)

        # ---- phase T: K table first (K-gathers can start while V builds) ----
        for m in range(N // 128):
            mb_, mi = divmod(m, 4)
            if mi == 0:
                xb = xp.tile([128, 2, 512], F16, tag="xb")
                nc.sync.dma_start(
                    out=xb[:],
                    in_=xT[:, bass.ts(mb_, 512)].rearrange("(c k) n -> k c n", c=2))
                k_st = tp.tile([128, 4, D], F16, tag="kst")
            if mi % 2 == 0:
                pk_ = psA.tile([128, 2, D], F32, space="PSUM", tag="pkb")
            nc.tensor.matmul(out=pk_[:, mi % 2, :], lhsT=xb[:, 0, bass.ts(mi, 128)],
                             rhs=wk_t[:, 0, :], start=True, stop=False)
            nc.tensor.matmul(out=pk_[:, mi % 2, :], lhsT=xb[:, 1, bass.ts(mi, 128)],
                             rhs=wk_t[:, 1, :], start=False, stop=True)
            if mi % 2 == 1:
                nc.scalar.copy(out=k_st[:, mi - 1:mi + 1, :], in_=pk_[:])
            if mi == 3:
                nc.sync.dma_start(
                    out=k_tab[bass.ts(mb_, 512), :].rearrange(
                        "(q p) d -> p q d", p=128), in_=k_st[:])

        # ---- phase T: V table ----
        for m in range(N // 128):
            mb_, mi = divmod(m, 4)
            if mi == 0:
                xb2 = xp.tile([128, 2, 512], F16, tag="xb2")
                nc.sync.dma_start(
                    out=xb2[:],
                    in_=xT[:, bass.ts(mb_, 512)].rearrange("(c k) n -> k c n", c=2))
                v_st = tp.tile([128, 4, D], F16, tag="vst")
            if mi % 2 == 0:
                pv_ = psA.tile([128, 2, D], F32, space="PSUM", tag="pvb")
            nc.tensor.matmul(out=pv_[:, mi % 2, :], lhsT=xb2[:, 0, bass.ts(mi, 128)],
                             rhs=wv_t[:, 0, :], start=True, stop=False)
            nc.tensor.matmul(out=pv_[:, mi % 2, :], lhsT=xb2[:, 1, bass.ts(mi, 128)],
                             rhs=wv_t[:, 1, :], start=False, stop=True)
            if mi % 2 == 1:
                nc.scalar.copy(out=v_st[:, mi - 1:mi + 1, :], in_=pv_[:])
            if mi == 3:
                nc.scalar.dma_start(
                    out=v_tab[bass.ts(mb_, 512), :].rearrange(
                        "(q p) d -> p q d", p=128), in_=v_st[:])

        # ---- phase A: software-pipelined attention ----
        # In-place halving trees inside the gathered tiles; z-tree on Pool.
        ctx_parts = [[None] * NU for _ in range(NT)]
        z_parts = [[None] * NU for _ in range(NT)]
        units = [(t, u) for t in range(NT) for u in range(NU)]
        NUNITS = len(units)
        kg_t, vg_t, s6_t, w_t = ({} for _ in range(4))

        def st_gk(i):
            t, u = units[i]
            kg = gkp.tile([128, UP, D], F16, tag="kg")
            nc.gpsimd.indirect_dma_start(
                out=kg[:], out_offset=None, in_=k_tab[:],
                in_offset=bass.IndirectOffsetOnAxis(
                    ap=pu_tiles[t][:, u * UP:(u + 1) * UP], axis=0))
            kg_t[i] = kg

        def st_gv(i):
            t, u = units[i]
            vg = gvp.tile([128, UP, D], F16, tag="vg")
            nc.gpsimd.indirect_dma_start(
                out=vg[:], out_offset=None, in_=v_tab[:],
                in_offset=bass.IndirectOffsetOnAxis(
                    ap=pu_tiles[t][:, u * UP:(u + 1) * UP], axis=0))
            vg_t[i] = vg

        def st_m1(i):
            t, u = units[i]
            kg = kg_t[i]
            nc.vector.tensor_tensor(
                out=kg[:], in0=kg[:],
                in1=q_tiles[t][:].unsqueeze(1).to_broadcast([128, UP, D]),
                op=MULT)

        def st_t1(i):
            kg = kg_t[i]
            pk = kg[:].rearrange("p u (e h) -> p u e h", h=H)
            nc.vector.tensor_tensor(out=pk[:, :, 0:16, :], in0=pk[:, :, 0:16, :],
                                    in1=pk[:, :, 16:32, :], op=ADD)

        def st_td(i):
            t, u = units[i]
            kg = kg_t.pop(i)
            pk = kg[:].rearrange("p u (e h) -> p u e h", h=H)
            nc.vector.tensor_tensor(out=pk[:, :, 0:8, :], in0=pk[:, :, 0:8, :],
                                    in1=pk[:, :, 8:16, :], op=ADD)
            nc.vector.tensor_tensor(out=pk[:, :, 0:4, :], in0=pk[:, :, 0:4, :],
                                    in1=pk[:, :, 4:8, :], op=ADD)
            nc.vector.tensor_tensor(out=pk[:, :, 0:2, :], in0=pk[:, :, 0:2, :],
                                    in1=pk[:, :, 2:4, :], op=ADD)
            nc.vector.tensor_tensor(out=pk[:, :, 0, :], in0=pk[:, :, 0, :],
                                    in1=pk[:, :, 1, :], op=ADD)
            s6 = sdp.tile([128, RP, CARD, H], F16, tag="s6")
            nc.vector.tensor_tensor(
                out=s6[:],
                in0=pk[:, :, 0, :].rearrange("p (r c) h -> p r c h", r=RP),
                in1=qek_tiles[t][:, u * RP:(u + 1) * RP, :].unsqueeze(2)
                    .to_broadcast([128, RP, CARD, H]),
                op=ADD)
            s6_t[i] = s6

        def st_e(i):
            s6 = s6_t.pop(i)
            w_u = wep.tile([128, RP, CARD, H], F16, tag="wu")
            nc.scalar.activation(out=w_u[:], in_=s6[:],
                                 func=mybir.ActivationFunctionType.Exp)
            w_t[i] = w_u

        def st_m2(i):
            vg = vg_t[i]
            w_u = w_t[i]
            nc.vector.tensor_tensor(
                out=vg[:].rearrange("p u (e h) -> p u e h", h=H),
                in0=vg[:].rearrange("p u (e h) -> p u e h", h=H),
                in1=w_u[:].rearrange("p r c h -> p (r c) h").unsqueeze(2)
                    .to_broadcast([128, UP, DH, H]),
                op=MULT)

        def st_c1(i):
            vg = vg_t[i]
            pv = vg[:].rearrange("p (r c) e -> p r c e", r=RP)
            nc.vector.tensor_tensor(out=pv[:, :, 0:4, :], in0=pv[:, :, 0:4, :],
                                    in1=pv[:, :, 4:8, :], op=ADD)

        def st_cd(i):
            t, u = units[i]
            vg = vg_t.pop(i)
            pv = vg[:].rearrange("p (r c) e -> p r c e", r=RP)
            nc.vector.tensor_tensor(out=pv[:, :, 0:2, :], in0=pv[:, :, 0:2, :],
                                    in1=pv[:, :, 2:4, :], op=ADD)
            nc.vector.tensor_tensor(out=pv[:, :, 0, :], in0=pv[:, :, 0, :],
                                    in1=pv[:, :, 1, :], op=ADD)
            nc.vector.tensor_tensor(out=pv[:, 0:2, 0, :], in0=pv[:, 0:2, 0, :],
                                    in1=pv[:, 2:4, 0, :], op=ADD)
            ctxu = cp.tile([128, D], F16, tag=f"ctxu{t}_{u}")
            nc.vector.tensor_tensor(out=ctxu[:], in0=pv[:, 0, 0, :],
                                    in1=pv[:, 1, 0, :], op=ADD)
            ctx_parts[t][u] = ctxu
            # z-tree on Pool (w_u stays intact for nothing else; reduce c)
            w_u = w_t.pop(i)
            nc.gpsimd.tensor_tensor(out=w_u[:, :, 0:4, :], in0=w_u[:, :, 0:4, :],
                                    in1=w_u[:, :, 4:8, :], op=ADD)
            nc.gpsimd.tensor_tensor(out=w_u[:, :, 0:2, :], in0=w_u[:, :, 0:2, :],
                                    in1=w_u[:, :, 2:4, :], op=ADD)
            zu = zp.tile([128, RP, H], F16, tag=f"zu{t}_{u}")
            nc.gpsimd.tensor_tensor(out=zu[:], in0=w_u[:, :, 0, :],
                                    in1=w_u[:, :, 1, :], op=ADD)
            z_parts[t][u] = zu

        # ---- phase O (emitted inside pipeline) ----
        def st_o(t):
                za = op_.tile([128, RP, H], F16, tag="za")
                nc.vector.tensor_tensor(out=za[:], in0=z_parts[t][0][:],
                                        in1=z_parts[t][1][:], op=ADD)
                nc.vector.tensor_tensor(out=za[:, 0:2, :], in0=za[:, 0:2, :],
                                        in1=za[:, 2:4, :], op=ADD)
                zs = op_.tile([128, H], F16, tag="zs")
                nc.vector.tensor_tensor(out=zs[:], in0=za[:, 0, :],
                                        in1=za[:, 1, :], op=ADD)
                zr = op_.tile([128, H], F16, tag="zr")
                nc.vector.reciprocal(out=zr[:], in_=zs[:])

                cs = op_.tile([128, D], F16, tag="cs")
                nc.vector.tensor_tensor(out=cs[:], in0=ctx_parts[t][0][:],
                                        in1=ctx_parts[t][1][:], op=ADD)
                ctxn = op_.tile([128, D], F32, tag="ctxn")
                nc.vector.tensor_tensor(
                    out=ctxn[:].rearrange("p (e h) -> p e h", h=H),
                    in0=cs[:].rearrange("p (e h) -> p e h", h=H),
                    in1=zr[:].unsqueeze(1).to_broadcast([128, DH, H]),
                    op=MULT)

                # transpose + out-proj
                ctxT = op_.tile([128, 2, 128], F16, tag="ctxT")
                for ch in range(2):
                    ptr = psC.tile([128, 128], F32, space="PSUM", tag="ptr")
                    nc.tensor.transpose(out=ptr[:], in_=ctxn[:, bass.ts(ch, 128)],
                                        identity=ident[:])
                    nc.scalar.copy(out=ctxT[:, ch, :], in_=ptr[:])
                po = psB.tile([128, D], F32, space="PSUM", tag="p256")
                nc.tensor.matmul(out=po[:], lhsT=ctxT[:, 0, :], rhs=owT_t[:, 0, :],
                                 start=True, stop=False)
                nc.tensor.matmul(out=po[:], lhsT=ctxT[:, 1, :], rhs=owT_t[:, 1, :],
                                 start=False, stop=True)
                o_sb = op_.tile([128, D], F32, tag="osb")
                nc.scalar.activation(out=o_sb[:], in_=po[:],
                                     func=mybir.ActivationFunctionType.Relu)
                nc.gpsimd.dma_start(out=out[bass.ts(t, 128), :], in_=o_sb[:])


        schedule = [(st_gk, 0), (st_gv, 1), (st_m1, 1), (st_t1, 2),
                    (st_td, 2), (st_e, 2), (st_m2, 3), (st_c1, 3), (st_cd, 4)]
        MAXLAG = max(lag for _, lag in schedule)
        for i in range(NUNITS + MAXLAG):
            for fn_, lag in schedule:
                j = i - lag
                if 0 <= j < NUNITS:
                    fn_(j)
            if (i - 4) % NU == NU - 1 and 0 <= (i - 4) < NUNITS:
                st_o((i - 4) // NU)

    return nc


# ---------------------------------------------------------------------------
def host_prep(x, incidence, edge_attr, W_lin, W_edge,
              in_proj_w, in_proj_b, out_proj_w, out_proj_b):
    x = np.asarray(x, np.float32)
    inc = np.asarray(incidence, np.float32)
    ea = np.asarray(edge_attr, np.float32)
    W_lin = np.asarray(W_lin, np.float32)
    W_edge = np.asarray(W_edge, np.float32)
    in_proj_w = np.asarray(in_proj_w, np.float32)
    in_proj_b = np.asarray(in_proj_b, np.float32)
    out_proj_w = np.asarray(out_proj_w, np.float32)
    out_proj_b = np.asarray(out_proj_b, np.float32)
    assert not in_proj_b.any() and not out_proj_b.any(), "nonzero bias unsupported"

    # index lists from incidence (order within a node's pair set is irrelevant:
    # attention is permutation-invariant over the L pairs)
    eon = np.nonzero(inc.T)[1].reshape(N, DEG).astype(np.int32)   # edge_of_node
    noe = np.nonzero(inc)[1].reshape(E, CARD).astype(np.int32)    # node_of_edge
    pair_u = noe[eon].astype(np.int32)                            # [N, DEG, CARD]
    # move the self pair (u == n) to slot c=0 of each round
    nn = np.arange(N)[:, None]
    self_c = np.argmax(pair_u == nn[:, :, None][..., 0:1] * 0 + nn[:, None, :]
                       if False else (pair_u == np.arange(N)[:, None, None]), axis=2)
    for r in range(DEG):
        c0 = self_c[:, r]
        row = pair_u[np.arange(N), r, c0].copy()
        pair_u[np.arange(N), r, c0] = pair_u[:, r, 0]
        pair_u[:, r, 0] = row
    assert (pair_u[:, :, 0] == np.arange(N)[:, None]).all()
    pair_u = pair_u.reshape(N, L)
    pair_e = eon

    Wq, Wk, Wv = in_proj_w[0:D], in_proj_w[D:2 * D], in_proj_w[2 * D:3 * D]
    scale = 1.0 / np.sqrt(np.float32(DH))

    # (e,h)-interleaved column order: new col e*H+h <- old col h*DH+e
    perm = np.arange(D).reshape(H, DH).T.reshape(-1)

    wkc = (W_lin @ Wk.T)[:, perm]
    wvc = (W_lin @ Wv.T)[:, perm]
    wqc = (W_lin @ Wq.T * scale)[:, perm]
    wekc = (W_edge @ Wk.T)[:, perm]
    owT = out_proj_w.T[perm, :].copy()

    import ml_dtypes
    f16 = ml_dtypes.bfloat16
    rep = dict(
        xT=np.ascontiguousarray(x.T).astype(f16),
        eaT=np.ascontiguousarray(ea.T).astype(f16),
        wk=wkc.astype(f16), wv=wvc.astype(f16), wq=wqc.astype(f16),
        wek=wekc.astype(f16), owT=owT.astype(f16),
    )
    per_core = []
    for c in range(NCORES):
        sl = slice(c * NSH, (c + 1) * NSH)
        m = dict(rep)
        m["xT_own"] = np.ascontiguousarray(x.T[:, sl]).astype(f16)
        m["pu"] = pair_u[sl]
        m["pe"] = pair_e[sl]
        per_core.append(m)
    return per_core


_CACHE = {}


def kernel(x, incidence, edge_attr, W_lin, W_edge,
           in_proj_w, in_proj_b, out_proj_w, out_proj_b, deg, card):
    assert int(deg) == DEG and int(card) == CARD
    in_maps = host_prep(x, incidence, edge_attr, W_lin, W_edge,
                        in_proj_w, in_proj_b, out_proj_w, out_proj_b)
    if "nc" not in _CACHE:
        _CACHE["nc"] = build_nc()
    from concourse.bass_utils import run_bass_kernel_spmd
    res = run_bass_kernel_spmd(_CACHE["nc"], in_maps, list(range(NCORES)))
    return np.concatenate([res.results[c]["out"] for c in range(NCORES)], axis=0)



# revision 21
# speedup vs baseline: 1.0244x; 1.0244x over previous
"""Trainium2 Bass kernel for CustomHyperSemanticMessagePassing.

Hypergraph multi-head attention message passing, N=4096 nodes, E=4096 edges,
DEG=CARD=8, D=256, H=8 heads. Sharding: data-parallel over nodes (512/core).

Per core:
  phase T: build K|V row table (fp16, (e,h)-interleaved columns) and EK table
           with PE matmuls from transposed inputs; q for own nodes.
  phase Q: gather per-node EK rows for all 8 rounds, compute qek[n,r,h].
  phase A: per (128-node tile, round-pair): one batched indirect DMA gathers
           16 K|V rows per node; scores via fp16 multiply + halving-tree
           reductions (DVE 2x mode), exp on Act, weighted V-sum via fp16
           multiply + tree (split across DVE/Pool).
  phase O: softmax-normalize, transpose, out-proj on PE, relu, store.

All elementwise tensors keep a 2-byte dtype with the innermost dim packed so
the DVE runs in its 2x performance mode; TensorReduce (no fast mode) is
replaced by log2 trees of TensorTensor adds.
"""
import numpy as np

import bass_rust
import orjson
import concourse.bass as bass
import concourse.tile as tile
import concourse.bass_utils as bass_utils
import concourse.bass2jax as bass2jax
from concourse import mybir
from concourse.masks import make_identity

F32 = mybir.dt.float32
F16 = mybir.dt.bfloat16
F8 = mybir.dt.float8e4
I32 = mybir.dt.int32

N, E, D, EDGE_DIM = 4096, 4096, 256, 64
H, DH, DEG, CARD = 8, 32, 8, 8
L = DEG * CARD
NCORES = 8
NSH = N // NCORES          # nodes per core
NT = NSH // 128            # 128-node tiles per core
RP = 4                     # rounds per gather unit
NU = DEG // RP             # units per tile
UP = RP * CARD             # pairs per unit

ADD = mybir.AluOpType.add
MULT = mybir.AluOpType.mult


# ---------------------------------------------------------------------------
# walrus workaround: this build accepts only one sync-wait per instruction;
# split extras into injected single-wait NoOps at the BIR-JSON level.
_ORIG_COMPILE = bass_utils.compile_bir_kernel
_ctr = [0]


def _split_multiwaits(bir_json: bytes) -> bytes:
    bir = orjson.loads(bir_json)
    changed = False
    for f in bir.get("functions", []):
        for blk in f.get("blocks", []):
            out = []
            for ins in blk.get("instructions", []):
                si = ins.get("sync_info")
                waits = (si or {}).get("on_wait") or []
                if len(waits) > 1 and ins.get("engine") not in (None, "Unassigned"):
                    changed = True
                    for w in waits[:-1]:
                        _ctr[0] += 1
                        out.append({
                            "debug": ins.get("debug"),
                            "engine": ins["engine"],
                            "ins": [], "outs": [],
                            "name": f"WSPLIT-{_ctr[0]}",
                            "opcode": "NoOp",
                            "sync_info": {"on_wait": [w], "on_update": []},
                        })
                    si["on_wait"] = waits[-1:]
                out.append(ins)
            blk["instructions"] = out
    return orjson.dumps(bir) if changed else bir_json


def _patched_compile(bir_json, tmpdir, neff_name="file.neff"):
    return _ORIG_COMPILE(_split_multiwaits(bytes(bir_json)), tmpdir,
                         neff_name=neff_name)


def _install_patch():
    bass_utils.compile_bir_kernel = _patched_compile
    bass2jax.compile_bir_kernel = _patched_compile


_install_patch()


# ---------------------------------------------------------------------------
def build_nc():
    nc = bass.Bass(num_devices=NCORES)
    # replicated inputs (fp16, transposed)
    xT = nc.declare_dram_parameter("xT", [D, N], F16, isOutput=False)
    eaT = nc.declare_dram_parameter("eaT", [EDGE_DIM, E], F16, isOutput=False)
    wk = nc.declare_dram_parameter("wk", [D, D], F16, isOutput=False)
    wv = nc.declare_dram_parameter("wv", [D, D], F16, isOutput=False)
    wq = nc.declare_dram_parameter("wq", [D, D], F16, isOutput=False)
    wek = nc.declare_dram_parameter("wek", [EDGE_DIM, D], F16, isOutput=False)
    owT = nc.declare_dram_parameter("owT", [D, D], F16, isOutput=False)
    # per-core inputs
    xT_own = nc.declare_dram_parameter("xT_own", [D, NSH], F16, isOutput=False)
    pu = nc.declare_dram_parameter("pu", [NSH, L], I32, isOutput=False)
    pe_ = nc.declare_dram_parameter("pe", [NSH, DEG], I32, isOutput=False)
    # output
    out = nc.declare_dram_parameter("out", [NSH, D], F32, isOutput=True)
    # internal tables
    k_tab = nc.dram_tensor("k_tab", [N, D], F16)
    v_tab = nc.dram_tensor("v_tab", [N, D], F16)
    ek_tab = nc.dram_tensor("ek_tab", [E, D], F16)

    import contextlib
    with tile.TileContext(nc) as tc, \
         nc.allow_low_precision(reason="fp16 trees validated vs reference"), \
         contextlib.ExitStack() as es:
        wp = es.enter_context(tc.tile_pool(name="wpool", bufs=1))
        xp = es.enter_context(tc.tile_pool(name="xpool", bufs=2))
        xsp = es.enter_context(tc.tile_pool(name="xspool", bufs=1))
        tp = es.enter_context(tc.tile_pool(name="tpool", bufs=2))
        qp = es.enter_context(tc.tile_pool(name="qpool", bufs=1))
        ip = es.enter_context(tc.tile_pool(name="ipool", bufs=1))
        gkp = es.enter_context(tc.tile_pool(name="gkpool", bufs=3))
        gvp = es.enter_context(tc.tile_pool(name="gvpool", bufs=3))
        sdp = es.enter_context(tc.tile_pool(name="sdpool", bufs=2))
        wep = es.enter_context(tc.tile_pool(name="wepool", bufs=4))
        cp = es.enter_context(tc.tile_pool(name="cpool", bufs=1))
        zp = es.enter_context(tc.tile_pool(name="zpool", bufs=1))
        op_ = es.enter_context(tc.tile_pool(name="opool", bufs=2))
        psA = es.enter_context(tc.tile_pool(name="psA", bufs=2, space="PSUM"))
        psB = es.enter_context(tc.tile_pool(name="psB", bufs=2, space="PSUM"))
        psE = es.enter_context(tc.tile_pool(name="psE", bufs=1, space="PSUM"))
        psC = es.enter_context(tc.tile_pool(name="psC", bufs=1, space="PSUM"))

        # ---- resident weights / inputs ----
        wk_t = wp.tile([128, 2, D], F16)
        nc.sync.dma_start(out=wk_t[:], in_=wk[:].rearrange("(c k) o -> k c o", c=2))
        wv_t = wp.tile([128, 2, D], F16)
        nc.sync.dma_start(out=wv_t[:], in_=wv[:].rearrange("(c k) o -> k c o", c=2))
        wq_t = wp.tile([128, 2, D], F16)
        nc.sync.dma_start(out=wq_t[:], in_=wq[:].rearrange("(c k) o -> k c o", c=2))
        wek_t = wp.tile([EDGE_DIM, D], F16)
        nc.sync.dma_start(out=wek_t[:], in_=wek[:])
        owT_t = wp.tile([128, 2, D], F16)
        nc.sync.dma_start(out=owT_t[:], in_=owT[:].rearrange("(c k) o -> k c o", c=2))
        ident = wp.tile([128, 128], F32)
        make_identity(nc, ident[:])

        ea_sb = xsp.tile([EDGE_DIM, E], F16, tag="easb")
        nc.sync.dma_start(out=ea_sb[:], in_=eaT[:])
        xo_sb = xsp.tile([128, 2, NSH], F16, tag="xosb")
        nc.sync.dma_start(out=xo_sb[:], in_=xT_own[:].rearrange("(c k) n -> k c n", c=2))

        # ---- full x in SBUF (shared by K and V table builds) ----
        x_sb = xsp.tile([128, 2, N], F16, tag="xsb")
        nc.sync.dma_start(out=x_sb[:], in_=xT[:].rearrange("(c k) n -> k c n", c=2))

        # ---- phase T: q for own nodes (resident, first so phase Q can run) --
        q_tiles = []
        for t in range(NT):
            pq = psB.tile([128, D], F32, space="PSUM", tag="p256")
            nc.tensor.matmul(out=pq[:], lhsT=xo_sb[:, 0, bass.ts(t, 128)],
                             rhs=wq_t[:, 0, :], start=True, stop=False)
            nc.tensor.matmul(out=pq[:], lhsT=xo_sb[:, 1, bass.ts(t, 128)],
                             rhs=wq_t[:, 1, :], start=False, stop=True)
            q_t = qp.tile([128, D], F16, tag=f"q{t}")
            nc.scalar.copy(out=q_t[:], in_=pq[:])
            q_tiles.append(q_t)

        # ---- phase T: EK table ----
        for m in range(E // 128):
            mb_, mi = divmod(m, 4)
            if mi % 2 == 0:
                pek = psE.tile([128, 2, D], F32, space="PSUM", tag="pek2")
            nc.tensor.matmul(out=pek[:, mi % 2, :], lhsT=ea_sb[:, bass.ts(m, 128)],
                             rhs=wek_t[:], start=True, stop=True)
            if mi == 0:
                ek_st = tp.tile([128, 4, D], F16, tag="ekst")
            if mi % 2 == 1:
                nc.scalar.copy(out=ek_st[:, mi - 1:mi + 1, :], in_=pek[:])
            if mi == 3:
                nc.sync.dma_start(
                    out=ek_tab[bass.ts(mb_, 512), :].rearrange(
                        "(q p) d -> p q d", p=128), in_=ek_st[:])

        # ---- phase T: K table (first: K-gathers gate the pipeline) ----
        for m in range(N // 128):
            mb_, mi = divmod(m, 4)
            if mi == 0:
                k_st = tp.tile([128, 4, D], F16, tag="kst")
            if mi % 2 == 0:
                pk_ = psA.tile([128, 2, D], F32, space="PSUM", tag="pkb")
            nc.tensor.matmul(out=pk_[:, mi % 2, :], lhsT=x_sb[:, 0, bass.ts(m, 128)],
                             rhs=wk_t[:, 0, :], start=True, stop=False)
            nc.tensor.matmul(out=pk_[:, mi % 2, :], lhsT=x_sb[:, 1, bass.ts(m, 128)],
                             rhs=wk_t[:, 1, :], start=False, stop=True)
            if mi % 2 == 1:
                nc.scalar.copy(out=k_st[:, mi - 1:mi + 1, :], in_=pk_[:])
            if mi == 3:
                nc.sync.dma_start(
                    out=k_tab[bass.ts(mb_, 512), :].rearrange(
                        "(q p) d -> p q d", p=128), in_=k_st[:])

        # ---- phase Q: per-tile round-edge EK gather + qek (DVE trees) ----
        pu_tiles, qek_tiles = [], []
        for t in range(NT):
            pu_t = ip.tile([128, L], I32, tag=f"put{t}")
            nc.sync.dma_start(out=pu_t[:], in_=pu[bass.ts(t, 128), :])
            pe_t = ip.tile([128, DEG], I32, tag=f"pet{t}")
            nc.sync.dma_start(out=pe_t[:], in_=pe_[bass.ts(t, 128), :])
            pu_tiles.append(pu_t)

            ekg = xp.tile([128, DEG, D], F16, tag="ekg")
            nc.gpsimd.indirect_dma_start(
                out=ekg[:], out_offset=None, in_=ek_tab[:],
                in_offset=bass.IndirectOffsetOnAxis(ap=pe_t[:], axis=0))
            nc.vector.tensor_tensor(
                out=ekg[:], in0=ekg[:],
                in1=q_tiles[t][:].unsqueeze(1).to_broadcast([128, DEG, D]),
                op=MULT)
            pv = ekg[:].rearrange("p r (e h) -> p r e h", h=H)
            nc.vector.tensor_tensor(out=pv[:, :, 0:16, :], in0=pv[:, :, 0:16, :],
                                    in1=pv[:, :, 16:32, :], op=ADD)
            nc.vector.tensor_tensor(out=pv[:, :, 0:8, :], in0=pv[:, :, 0:8, :],
                                    in1=pv[:, :, 8:16, :], op=ADD)
            nc.vector.tensor_tensor(out=pv[:, :, 0:4, :], in0=pv[:, :, 0:4, :],
                                    in1=pv[:, :, 4:8, :], op=ADD)
            nc.vector.tensor_tensor(out=pv[:, :, 0:2, :], in0=pv[:, :, 0:2, :],
                                    in1=pv[:, :, 2:4, :], op=ADD)
            qek_t = qp.tile([128, DEG, H], F16, tag=f"qek{t}")
            nc.vector.tensor_tensor(out=qek_t[:], in0=pv[:, :, 0, :],
                                    in1=pv[:, :, 1, :], op=ADD)
            qek_tiles.append(qek_t)

        # ---- phase T: V table ----
        for m in range(N // 128):
            mb_, mi = divmod(m, 4)
            if mi == 0:
                v_st = tp.tile([128, 4, D], F16, tag="vst")
            if mi % 2 == 0:
                pv_ = psA.tile([128, 2, D], F32, space="PSUM", tag="pvb")
            nc.tensor.matmul(out=pv_[:, mi % 2, :], lhsT=x_sb[:, 0, bass.ts(m, 128)],
                             rhs=wv_t[:, 0, :], start=True, stop=False)
            nc.tensor.matmul(out=pv_[:, mi % 2, :], lhsT=x_sb[:, 1, bass.ts(m, 128)],
                             rhs=wv_t[:, 1, :], start=False, stop=True)
            if mi % 2 == 1:
                nc.scalar.copy(out=v_st[:, mi - 1:mi + 1, :], in_=pv_[:])
            if mi == 3:
                nc.sync.dma_start(
                    out=v_tab[bass.ts(mb_, 512), :].rearrange(
                        "(q p) d -> p q d", p=128), in_=v_st[:])

        # ---- phase A: software-pipelined attention ----
        # In-place halving trees inside the gathered tiles; z-tree on Pool.
        ctx_parts = [[None] * NU for _ in range(NT)]
        z_parts = [[None] * NU for _ in range(NT)]
        units = [(t, u) for t in range(NT) for u in range(NU)]
        NUNITS = len(units)
        kg_t, vg_t, s6_t, w_t = ({} for _ in range(4))

        def st_gk(i):
            t, u = units[i]
            kg = gkp.tile([128, UP, D], F16, tag="kg")
            nc.gpsimd.indirect_dma_start(
                out=kg[:], out_offset=None, in_=k_tab[:],
                in_offset=bass.IndirectOffsetOnAxis(
                    ap=pu_tiles[t][:, u * UP:(u + 1) * UP], axis=0))
            kg_t[i] = kg

        def st_gv(i):
            t, u = units[i]
            vg = gvp.tile([128, UP, D], F16, tag="vg")
            nc.gpsimd.indirect_dma_start(
                out=vg[:], out_offset=None, in_=v_tab[:],
                in_offset=bass.IndirectOffsetOnAxis(
                    ap=pu_tiles[t][:, u * UP:(u + 1) * UP], axis=0))
            vg_t[i] = vg

        def st_m1(i):
            t, u = units[i]
            kg = kg_t[i]
            nc.vector.tensor_tensor(
                out=kg[:], in0=kg[:],
                in1=q_tiles[t][:].unsqueeze(1).to_broadcast([128, UP, D]),
                op=MULT)

        def st_t1(i):
            kg = kg_t[i]
            pk = kg[:].rearrange("p u (e h) -> p u e h", h=H)
            nc.vector.tensor_tensor(out=pk[:, :, 0:16, :], in0=pk[:, :, 0:16, :],
                                    in1=pk[:, :, 16:32, :], op=ADD)

        def st_td(i):
            t, u = units[i]
            kg = kg_t.pop(i)
            pk = kg[:].rearrange("p u (e h) -> p u e h", h=H)
            nc.vector.tensor_tensor(out=pk[:, :, 0:8, :], in0=pk[:, :, 0:8, :],
                                    in1=pk[:, :, 8:16, :], op=ADD)
            nc.vector.tensor_tensor(out=pk[:, :, 0:4, :], in0=pk[:, :, 0:4, :],
                                    in1=pk[:, :, 4:8, :], op=ADD)
            nc.vector.tensor_tensor(out=pk[:, :, 0:2, :], in0=pk[:, :, 0:2, :],
                                    in1=pk[:, :, 2:4, :], op=ADD)
            nc.vector.tensor_tensor(out=pk[:, :, 0, :], in0=pk[:, :, 0, :],
                                    in1=pk[:, :, 1, :], op=ADD)
            s6 = sdp.tile([128, RP, CARD, H], F16, tag="s6")
            nc.vector.tensor_tensor(
                out=s6[:],
                in0=pk[:, :, 0, :].rearrange("p (r c) h -> p r c h", r=RP),
                in1=qek_tiles[t][:, u * RP:(u + 1) * RP, :].unsqueeze(2)
                    .to_broadcast([128, RP, CARD, H]),
                op=ADD)
            s6_t[i] = s6

        def st_e(i):
            s6 = s6_t.pop(i)
            w_u = wep.tile([128, RP, CARD, H], F16, tag="wu")
            nc.scalar.activation(out=w_u[:], in_=s6[:],
                                 func=mybir.ActivationFunctionType.Exp)
            w_t[i] = w_u

        def st_m2(i):
            vg = vg_t[i]
            w_u = w_t[i]
            nc.vector.tensor_tensor(
                out=vg[:].rearrange("p u (e h) -> p u e h", h=H),
                in0=vg[:].rearrange("p u (e h) -> p u e h", h=H),
                in1=w_u[:].rearrange("p r c h -> p (r c) h").unsqueeze(2)
                    .to_broadcast([128, UP, DH, H]),
                op=MULT)

        def st_c1(i):
            vg = vg_t[i]
            pv = vg[:].rearrange("p (r c) e -> p r c e", r=RP)
            nc.vector.tensor_tensor(out=pv[:, :, 0:4, :], in0=pv[:, :, 0:4, :],
                                    in1=pv[:, :, 4:8, :], op=ADD)

        def st_cd(i):
            t, u = units[i]
            vg = vg_t.pop(i)
            pv = vg[:].rearrange("p (r c) e -> p r c e", r=RP)
            nc.vector.tensor_tensor(out=pv[:, :, 0:2, :], in0=pv[:, :, 0:2, :],
                                    in1=pv[:, :, 2:4, :], op=ADD)
            nc.vector.tensor_tensor(out=pv[:, :, 0, :], in0=pv[:, :, 0, :],
                                    in1=pv[:, :, 1, :], op=ADD)
            nc.vector.tensor_tensor(out=pv[:, 0:2, 0, :], in0=pv[:, 0:2, 0, :],
                                    in1=pv[:, 2:4, 0, :], op=ADD)
            ctxu = cp.tile([128, D], F16, tag=f"ctxu{t}_{u}")
            nc.vector.tensor_tensor(out=ctxu[:], in0=pv[:, 0, 0, :],
                                    in1=pv[:, 1, 0, :], op=ADD)
            ctx_parts[t][u] = ctxu
            # z-tree on Pool (w_u stays intact for nothing else; reduce c)
            w_u = w_t.pop(i)
            nc.gpsimd.tensor_tensor(out=w_u[:, :, 0:4, :], in0=w_u[:, :, 0:4, :],
                                    in1=w_u[:, :, 4:8, :], op=ADD)
            nc.gpsimd.tensor_tensor(out=w_u[:, :, 0:2, :], in0=w_u[:, :, 0:2, :],
                                    in1=w_u[:, :, 2:4, :], op=ADD)
            zu = zp.tile([128, RP, H], F16, tag=f"zu{t}_{u}")
            nc.gpsimd.tensor_tensor(out=zu[:], in0=w_u[:, :, 0, :],
                                    in1=w_u[:, :, 1, :], op=ADD)
            z_parts[t][u] = zu

        # ---- phase O (emitted inside pipeline) ----
        def st_o(t):
                za = op_.tile([128, RP, H], F16, tag="za")
                nc.vector.tensor_tensor(out=za[:], in0=z_parts[t][0][:],
                                        in1=z_parts[t][1][:], op=ADD)
                nc.vector.tensor_tensor(out=za[:, 0:2, :], in0=za[:, 0:2, :],
                                        in1=za[:, 2:4, :], op=ADD)
                zs = op_.tile([128, H], F16, tag="zs")
                nc.vector.tensor_tensor(out=zs[:], in0=za[:, 0, :],
                                        in1=za[:, 1, :], op=ADD)
                zr = op_.tile([128, H], F16, tag="zr")
                nc.vector.reciprocal(out=zr[:], in_=zs[:])

                cs = op_.tile([128, D], F16, tag="cs")
                nc.vector.tensor_tensor(out=cs[:], in0=ctx_parts[t][0][:],
                                        in1=ctx_parts[t][1][:], op=ADD)
                ctxn = op_.tile([128, D], F32, tag="ctxn")
                nc.vector.tensor_tensor(
                    out=ctxn[:].rearrange("p (e h) -> p e h", h=H),
                    in0=cs[:].rearrange("p (e h) -> p e h", h=H),
                    in1=zr[:].unsqueeze(1).to_broadcast([128, DH, H]),
                    op=MULT)

                # transpose + out-proj
                ctxT = op_.tile([128, 2, 128], F16, tag="ctxT")
                for ch in range(2):
                    ptr = psC.tile([128, 128], F32, space="PSUM", tag="ptr")
                    nc.tensor.transpose(out=ptr[:], in_=ctxn[:, bass.ts(ch, 128)],
                                        identity=ident[:])
                    nc.scalar.copy(out=ctxT[:, ch, :], in_=ptr[:])
                po = psB.tile([128, D], F32, space="PSUM", tag="p256")
                nc.tensor.matmul(out=po[:], lhsT=ctxT[:, 0, :], rhs=owT_t[:, 0, :],
                                 start=True, stop=False)
                nc.tensor.matmul(out=po[:], lhsT=ctxT[:, 1, :], rhs=owT_t[:, 1, :],
                                 start=False, stop=True)
                o_sb = op_.tile([128, D], F32, tag="osb")
                nc.scalar.activation(out=o_sb[:], in_=po[:],
                                     func=mybir.ActivationFunctionType.Relu)
                nc.sync.dma_start(out=out[bass.ts(t, 128), :], in_=o_sb[:])


        schedule = [(st_gk, 0), (st_gv, 1), (st_m1, 1), (st_t1, 2),
                    (st_td, 2), (st_e, 2), (st_m2, 3), (st_c1, 3), (st_cd, 4)]
        MAXLAG = max(lag for _, lag in schedule)
        for i in range(NUNITS + MAXLAG):
            for fn_, lag in schedule:
                j = i - lag
                if 0 <= j < NUNITS:
                    fn_(j)
            if (i - 4) % NU == NU - 1 and 0 <= (i - 4) < NUNITS:
                st_o((i - 4) // NU)

    return nc


# ---------------------------------------------------------------------------
def host_prep(x, incidence, edge_attr, W_lin, W_edge,
              in_proj_w, in_proj_b, out_proj_w, out_proj_b):
    x = np.asarray(x, np.float32)
    inc = np.asarray(incidence, np.float32)
    ea = np.asarray(edge_attr, np.float32)
    W_lin = np.asarray(W_lin, np.float32)
    W_edge = np.asarray(W_edge, np.float32)
    in_proj_w = np.asarray(in_proj_w, np.float32)
    in_proj_b = np.asarray(in_proj_b, np.float32)
    out_proj_w = np.asarray(out_proj_w, np.float32)
    out_proj_b = np.asarray(out_proj_b, np.float32)
    assert not in_proj_b.any() and not out_proj_b.any(), "nonzero bias unsupported"

    # index lists from incidence (order within a node's pair set is irrelevant:
    # attention is permutation-invariant over the L pairs)
    eon = np.nonzero(inc.T)[1].reshape(N, DEG).astype(np.int32)   # edge_of_node
    noe = np.nonzero(inc)[1].reshape(E, CARD).astype(np.int32)    # node_of_edge
    pair_u = noe[eon].astype(np.int32)                            # [N, DEG, CARD]
    # move the self pair (u == n) to slot c=0 of each round
    nn = np.arange(N)[:, None]
    self_c = np.argmax(pair_u == nn[:, :, None][..., 0:1] * 0 + nn[:, None, :]
                       if False else (pair_u == np.arange(N)[:, None, None]), axis=2)
    for r in range(DEG):
        c0 = self_c[:, r]
        row = pair_u[np.arange(N), r, c0].copy()
        pair_u[np.arange(N), r, c0] = pair_u[:, r, 0]
        pair_u[:, r, 0] = row
    assert (pair_u[:, :, 0] == np.arange(N)[:, None]).all()
    pair_u = pair_u.reshape(N, L)
    pair_e = eon

    Wq, Wk, Wv = in_proj_w[0:D], in_proj_w[D:2 * D], in_proj_w[2 * D:3 * D]
    scale = 1.0 / np.sqrt(np.float32(DH))

    # (e,h)-interleaved column order: new col e*H+h <- old col h*DH+e
    perm = np.arange(D).reshape(H, DH).T.reshape(-1)

    wkc = (W_lin @ Wk.T)[:, perm]
    wvc = (W_lin @ Wv.T)[:, perm]
    wqc = (W_lin @ Wq.T * scale)[:, perm]
    wekc = (W_edge @ Wk.T)[:, perm]
    owT = out_proj_w.T[perm, :].copy()

    import ml_dtypes
    f16 = ml_dtypes.bfloat16
    rep = dict(
        xT=np.ascontiguousarray(x.T).astype(f16),
        eaT=np.ascontiguousarray(ea.T).astype(f16),
        wk=wkc.astype(f16), wv=wvc.astype(f16), wq=wqc.astype(f16),
        wek=wekc.astype(f16), owT=owT.astype(f16),
    )
    per_core = []
    for c in range(NCORES):
        sl = slice(c * NSH, (c + 1) * NSH)
        m = dict(rep)
        m["xT_own"] = np.ascontiguousarray(x.T[:, sl]).astype(f16)
        m["pu"] = pair_u[sl]
        m["pe"] = pair_e[sl]
        per_core.append(m)
    return per_core


_CACHE = {}


def kernel(x, incidence, edge_attr, W_lin, W_edge,
           in_proj_w, in_proj_b, out_proj_w, out_proj_b, deg, card):
    assert int(deg) == DEG and int(card) == CARD
    in_maps = host_prep(x, incidence, edge_attr, W_lin, W_edge,
                        in_proj_w, in_proj_b, out_proj_w, out_proj_b)
    if "nc" not in _CACHE:
        _CACHE["nc"] = build_nc()
    from concourse.bass_utils import run_bass_kernel_spmd
    res = run_bass_kernel_spmd(_CACHE["nc"], in_maps, list(range(NCORES)))
    return np.concatenate([res.results[c]["out"] for c in range(NCORES)], axis=0)



# revision 25
# speedup vs baseline: 1.0272x; 1.0027x over previous
"""Trainium2 Bass kernel for CustomHyperSemanticMessagePassing.

Hypergraph multi-head attention message passing, N=4096 nodes, E=4096 edges,
DEG=CARD=8, D=256, H=8 heads. Sharding: data-parallel over nodes (512/core).

Per core:
  phase T: build K|V row table (fp16, (e,h)-interleaved columns) and EK table
           with PE matmuls from transposed inputs; q for own nodes.
  phase Q: gather per-node EK rows for all 8 rounds, compute qek[n,r,h].
  phase A: per (128-node tile, round-pair): one batched indirect DMA gathers
           16 K|V rows per node; scores via fp16 multiply + halving-tree
           reductions (DVE 2x mode), exp on Act, weighted V-sum via fp16
           multiply + tree (split across DVE/Pool).
  phase O: softmax-normalize, transpose, out-proj on PE, relu, store.

All elementwise tensors keep a 2-byte dtype with the innermost dim packed so
the DVE runs in its 2x performance mode; TensorReduce (no fast mode) is
replaced by log2 trees of TensorTensor adds.
"""
import numpy as np

import bass_rust
import orjson
import concourse.bass as bass
import concourse.tile as tile
import concourse.bass_utils as bass_utils
import concourse.bass2jax as bass2jax
from concourse import mybir
from concourse.masks import make_identity

F32 = mybir.dt.float32
F16 = mybir.dt.bfloat16
F8 = mybir.dt.float8e4
I32 = mybir.dt.int32

N, E, D, EDGE_DIM = 4096, 4096, 256, 64
H, DH, DEG, CARD = 8, 32, 8, 8
L = DEG * CARD
NCORES = 8
NSH = N // NCORES          # nodes per core
NT = NSH // 128            # 128-node tiles per core
RP = 4                     # rounds per gather unit
NU = DEG // RP             # units per tile
UP = RP * CARD             # pairs per unit

ADD = mybir.AluOpType.add
MULT = mybir.AluOpType.mult


# ---------------------------------------------------------------------------
# walrus workaround: this build accepts only one sync-wait per instruction;
# split extras into injected single-wait NoOps at the BIR-JSON level.
_ORIG_COMPILE = bass_utils.compile_bir_kernel
_ctr = [0]


def _split_multiwaits(bir_json: bytes) -> bytes:
    bir = orjson.loads(bir_json)
    changed = False
    for f in bir.get("functions", []):
        for blk in f.get("blocks", []):
            out = []
            for ins in blk.get("instructions", []):
                si = ins.get("sync_info")
                waits = (si or {}).get("on_wait") or []
                if len(waits) > 1 and ins.get("engine") not in (None, "Unassigned"):
                    changed = True
                    for w in waits[:-1]:
                        _ctr[0] += 1
                        out.append({
                            "debug": ins.get("debug"),
                            "engine": ins["engine"],
                            "ins": [], "outs": [],
                            "name": f"WSPLIT-{_ctr[0]}",
                            "opcode": "NoOp",
                            "sync_info": {"on_wait": [w], "on_update": []},
                        })
                    si["on_wait"] = waits[-1:]
                out.append(ins)
            blk["instructions"] = out
    return orjson.dumps(bir) if changed else bir_json


def _patched_compile(bir_json, tmpdir, neff_name="file.neff"):
    return _ORIG_COMPILE(_split_multiwaits(bytes(bir_json)), tmpdir,
                         neff_name=neff_name)


def _install_patch():
    bass_utils.compile_bir_kernel = _patched_compile
    bass2jax.compile_bir_kernel = _patched_compile


_install_patch()


# ---------------------------------------------------------------------------
def build_nc():
    nc = bass.Bass(num_devices=NCORES)
    # replicated inputs (fp16, transposed)
    xT = nc.declare_dram_parameter("xT", [D, N], F16, isOutput=False)
    eaT = nc.declare_dram_parameter("eaT", [EDGE_DIM, E], F16, isOutput=False)
    wk = nc.declare_dram_parameter("wk", [D, D], F16, isOutput=False)
    wv = nc.declare_dram_parameter("wv", [D, D], F16, isOutput=False)
    wq = nc.declare_dram_parameter("wq", [D, D], F16, isOutput=False)
    wek = nc.declare_dram_parameter("wek", [EDGE_DIM, D], F16, isOutput=False)
    owT = nc.declare_dram_parameter("owT", [D, D], F16, isOutput=False)
    # per-core inputs
    xT_own = nc.declare_dram_parameter("xT_own", [D, NSH], F16, isOutput=False)
    pu = nc.declare_dram_parameter("pu", [NSH, L], I32, isOutput=False)
    pe_ = nc.declare_dram_parameter("pe", [NSH, DEG], I32, isOutput=False)
    # output
    out = nc.declare_dram_parameter("out", [NSH, D], F32, isOutput=True)
    # internal tables
    k_tab = nc.dram_tensor("k_tab", [N, D], F16)
    v_tab = nc.dram_tensor("v_tab", [N, D], F16)
    ek_tab = nc.dram_tensor("ek_tab", [E, D], F16)

    import contextlib
    with tile.TileContext(nc) as tc, \
         nc.allow_low_precision(reason="fp16 trees validated vs reference"), \
         contextlib.ExitStack() as es:
        wp = es.enter_context(tc.tile_pool(name="wpool", bufs=1))
        xp = es.enter_context(tc.tile_pool(name="xpool", bufs=2))
        xsp = es.enter_context(tc.tile_pool(name="xspool", bufs=1))
        tp = es.enter_context(tc.tile_pool(name="tpool", bufs=2))
        qp = es.enter_context(tc.tile_pool(name="qpool", bufs=1))
        ip = es.enter_context(tc.tile_pool(name="ipool", bufs=1))
        gkp = es.enter_context(tc.tile_pool(name="gkpool", bufs=3))
        gvp = es.enter_context(tc.tile_pool(name="gvpool", bufs=3))
        sdp = es.enter_context(tc.tile_pool(name="sdpool", bufs=2))
        wep = es.enter_context(tc.tile_pool(name="wepool", bufs=4))
        cp = es.enter_context(tc.tile_pool(name="cpool", bufs=1))
        zp = es.enter_context(tc.tile_pool(name="zpool", bufs=1))
        op_ = es.enter_context(tc.tile_pool(name="opool", bufs=2))
        psA = es.enter_context(tc.tile_pool(name="psA", bufs=2, space="PSUM"))
        psB = es.enter_context(tc.tile_pool(name="psB", bufs=2, space="PSUM"))
        psE = es.enter_context(tc.tile_pool(name="psE", bufs=1, space="PSUM"))
        psC = es.enter_context(tc.tile_pool(name="psC", bufs=1, space="PSUM"))

        # ---- resident weights / inputs ----
        wk_t = wp.tile([128, 2, D], F16)
        nc.sync.dma_start(out=wk_t[:], in_=wk[:].rearrange("(c k) o -> k c o", c=2))
        wv_t = wp.tile([128, 2, D], F16)
        nc.sync.dma_start(out=wv_t[:], in_=wv[:].rearrange("(c k) o -> k c o", c=2))
        wq_t = wp.tile([128, 2, D], F16)
        nc.sync.dma_start(out=wq_t[:], in_=wq[:].rearrange("(c k) o -> k c o", c=2))
        wek_t = wp.tile([EDGE_DIM, D], F16)
        nc.sync.dma_start(out=wek_t[:], in_=wek[:])
        owT_t = wp.tile([128, 2, D], F16)
        nc.sync.dma_start(out=owT_t[:], in_=owT[:].rearrange("(c k) o -> k c o", c=2))
        ident = wp.tile([128, 128], F32)
        make_identity(nc, ident[:])

        xo_sb = xsp.tile([128, 2, NSH], F16, tag="xosb")
        nc.sync.dma_start(out=xo_sb[:], in_=xT_own[:].rearrange("(c k) n -> k c n", c=2))

        # ---- full x in SBUF, 4 chunks so K matmuls start early ----
        XC = N // 4
        x_sbs = []
        for j in range(4):
            x_sbj = xsp.tile([128, 2, XC], F16, tag=f"xsb{j}")
            nc.sync.dma_start(
                out=x_sbj[:],
                in_=xT[:, bass.ts(j, XC)].rearrange("(c k) n -> k c n", c=2))
            x_sbs.append(x_sbj)

        # ---- pair/edge index loads (no deps — issue early) ----
        pu_tiles, pe_tiles = [], []
        for t in range(NT):
            pu_t = ip.tile([128, L], I32, tag=f"put{t}")
            nc.sync.dma_start(out=pu_t[:], in_=pu[bass.ts(t, 128), :])
            pe_t = ip.tile([128, DEG], I32, tag=f"pet{t}")
            nc.sync.dma_start(out=pe_t[:], in_=pe_[bass.ts(t, 128), :])
            pu_tiles.append(pu_t)
            pe_tiles.append(pe_t)

        ea_sb = xsp.tile([EDGE_DIM, E], F16, tag="easb")
        nc.sync.dma_start(out=ea_sb[:], in_=eaT[:])

        # ---- phase T: q for own nodes (resident, first so phase Q can run) --
        q_tiles = []
        for t in range(NT):
            pq = psB.tile([128, D], F32, space="PSUM", tag="p256")
            nc.tensor.matmul(out=pq[:], lhsT=xo_sb[:, 0, bass.ts(t, 128)],
                             rhs=wq_t[:, 0, :], start=True, stop=False)
            nc.tensor.matmul(out=pq[:], lhsT=xo_sb[:, 1, bass.ts(t, 128)],
                             rhs=wq_t[:, 1, :], start=False, stop=True)
            q_t = qp.tile([128, D], F16, tag=f"q{t}")
            nc.scalar.copy(out=q_t[:], in_=pq[:])
            q_tiles.append(q_t)

        # ---- phase T: EK table ----
        for m in range(E // 128):
            mb_, mi = divmod(m, 4)
            if mi % 2 == 0:
                pek = psE.tile([128, 2, D], F32, space="PSUM", tag="pek2")
            nc.tensor.matmul(out=pek[:, mi % 2, :], lhsT=ea_sb[:, bass.ts(m, 128)],
                             rhs=wek_t[:], start=True, stop=True)
            if mi == 0:
                ek_st = tp.tile([128, 4, D], F16, tag="ekst")
            if mi % 2 == 1:
                nc.scalar.copy(out=ek_st[:, mi - 1:mi + 1, :], in_=pek[:])
            if mi == 3:
                nc.gpsimd.dma_start(
                    out=ek_tab[bass.ts(mb_, 512), :].rearrange(
                        "(q p) d -> p q d", p=128), in_=ek_st[:])

        # ---- phase T: K table (first: K-gathers gate the pipeline) ----
        for m in range(N // 128):
            mb_, mi = divmod(m, 4)
            xj = x_sbs[m // 8]
            mj = m % 8
            if mi == 0:
                k_st = tp.tile([128, 4, D], F16, tag="kst")
            if mi % 2 == 0:
                pk_ = psA.tile([128, 2, D], F32, space="PSUM", tag="pkb")
            nc.tensor.matmul(out=pk_[:, mi % 2, :], lhsT=xj[:, 0, bass.ts(mj, 128)],
                             rhs=wk_t[:, 0, :], start=True, stop=False)
            nc.tensor.matmul(out=pk_[:, mi % 2, :], lhsT=xj[:, 1, bass.ts(mj, 128)],
                             rhs=wk_t[:, 1, :], start=False, stop=True)
            if mi % 2 == 1:
                nc.scalar.copy(out=k_st[:, mi - 1:mi + 1, :], in_=pk_[:])
            if mi == 3:
                nc.gpsimd.dma_start(
                    out=k_tab[bass.ts(mb_, 512), :].rearrange(
                        "(q p) d -> p q d", p=128), in_=k_st[:])

        # ---- phase Q: per-tile round-edge EK gather + qek (DVE trees) ----
        qek_tiles = []
        for t in range(NT):
            pe_t = pe_tiles[t]
            ekg = xp.tile([128, DEG, D], F16, tag="ekg")
            nc.gpsimd.indirect_dma_start(
                out=ekg[:], out_offset=None, in_=ek_tab[:],
                in_offset=bass.IndirectOffsetOnAxis(ap=pe_t[:], axis=0))
            nc.vector.tensor_tensor(
                out=ekg[:], in0=ekg[:],
                in1=q_tiles[t][:].unsqueeze(1).to_broadcast([128, DEG, D]),
                op=MULT)
            pv = ekg[:].rearrange("p r (e h) -> p r e h", h=H)
            nc.vector.tensor_tensor(out=pv[:, :, 0:16, :], in0=pv[:, :, 0:16, :],
                                    in1=pv[:, :, 16:32, :], op=ADD)
            nc.vector.tensor_tensor(out=pv[:, :, 0:8, :], in0=pv[:, :, 0:8, :],
                                    in1=pv[:, :, 8:16, :], op=ADD)
            nc.vector.tensor_tensor(out=pv[:, :, 0:4, :], in0=pv[:, :, 0:4, :],
                                    in1=pv[:, :, 4:8, :], op=ADD)
            nc.vector.tensor_tensor(out=pv[:, :, 0:2, :], in0=pv[:, :, 0:2, :],
                                    in1=pv[:, :, 2:4, :], op=ADD)
            qek_t = qp.tile([128, DEG, H], F16, tag=f"qek{t}")
            nc.vector.tensor_tensor(out=qek_t[:], in0=pv[:, :, 0, :],
                                    in1=pv[:, :, 1, :], op=ADD)
            qek_tiles.append(qek_t)

        # ---- phase T: V table ----
        for m in range(N // 128):
            mb_, mi = divmod(m, 4)
            xj = x_sbs[m // 8]
            mj = m % 8
            if mi == 0:
                v_st = tp.tile([128, 4, D], F16, tag="vst")
            if mi % 2 == 0:
                pv_ = psA.tile([128, 2, D], F32, space="PSUM", tag="pvb")
            nc.tensor.matmul(out=pv_[:, mi % 2, :], lhsT=xj[:, 0, bass.ts(mj, 128)],
                             rhs=wv_t[:, 0, :], start=True, stop=False)
            nc.tensor.matmul(out=pv_[:, mi % 2, :], lhsT=xj[:, 1, bass.ts(mj, 128)],
                             rhs=wv_t[:, 1, :], start=False, stop=True)
            if mi % 2 == 1:
                nc.scalar.copy(out=v_st[:, mi - 1:mi + 1, :], in_=pv_[:])
            if mi == 3:
                nc.sync.dma_start(
                    out=v_tab[bass.ts(mb_, 512), :].rearrange(
                        "(q p) d -> p q d", p=128), in_=v_st[:])

        # ---- phase A: software-pipelined attention ----
        # In-place halving trees inside the gathered tiles; z-tree on Pool.
        ctx_parts = [[None] * NU for _ in range(NT)]
        z_parts = [[None] * NU for _ in range(NT)]
        units = [(t, u) for t in range(NT) for u in range(NU)]
        NUNITS = len(units)
        kg_t, vg_t, s6_t, w_t = ({} for _ in range(4))

        def st_gk(i):
            t, u = units[i]
            kg = gkp.tile([128, UP, D], F16, tag="kg")
            nc.gpsimd.indirect_dma_start(
                out=kg[:], out_offset=None, in_=k_tab[:],
                in_offset=bass.IndirectOffsetOnAxis(
                    ap=pu_tiles[t][:, u * UP:(u + 1) * UP], axis=0))
            kg_t[i] = kg

        def st_gv(i):
            t, u = units[i]
            vg = gvp.tile([128, UP, D], F16, tag="vg")
            nc.gpsimd.indirect_dma_start(
                out=vg[:], out_offset=None, in_=v_tab[:],
                in_offset=bass.IndirectOffsetOnAxis(
                    ap=pu_tiles[t][:, u * UP:(u + 1) * UP], axis=0))
            vg_t[i] = vg

        def st_m1(i):
            t, u = units[i]
            kg = kg_t[i]
            nc.vector.tensor_tensor(
                out=kg[:], in0=kg[:],
                in1=q_tiles[t][:].unsqueeze(1).to_broadcast([128, UP, D]),
                op=MULT)

        def st_t1(i):
            kg = kg_t[i]
            pk = kg[:].rearrange("p u (e h) -> p u e h", h=H)
            nc.vector.tensor_tensor(out=pk[:, :, 0:16, :], in0=pk[:, :, 0:16, :],
                                    in1=pk[:, :, 16:32, :], op=ADD)

        def st_td(i):
            t, u = units[i]
            kg = kg_t.pop(i)
            pk = kg[:].rearrange("p u (e h) -> p u e h", h=H)
            nc.vector.tensor_tensor(out=pk[:, :, 0:8, :], in0=pk[:, :, 0:8, :],
                                    in1=pk[:, :, 8:16, :], op=ADD)
            nc.vector.tensor_tensor(out=pk[:, :, 0:4, :], in0=pk[:, :, 0:4, :],
                                    in1=pk[:, :, 4:8, :], op=ADD)
            nc.vector.tensor_tensor(out=pk[:, :, 0:2, :], in0=pk[:, :, 0:2, :],
                                    in1=pk[:, :, 2:4, :], op=ADD)
            nc.vector.tensor_tensor(out=pk[:, :, 0, :], in0=pk[:, :, 0, :],
                                    in1=pk[:, :, 1, :], op=ADD)
            s6 = sdp.tile([128, RP, CARD, H], F16, tag="s6")
            nc.vector.tensor_tensor(
                out=s6[:],
                in0=pk[:, :, 0, :].rearrange("p (r c) h -> p r c h", r=RP),
                in1=qek_tiles[t][:, u * RP:(u + 1) * RP, :].unsqueeze(2)
                    .to_broadcast([128, RP, CARD, H]),
                op=ADD)
            s6_t[i] = s6

        def st_e(i):
            s6 = s6_t.pop(i)
            w_u = wep.tile([128, RP, CARD, H], F16, tag="wu")
            nc.scalar.activation(out=w_u[:], in_=s6[:],
                                 func=mybir.ActivationFunctionType.Exp)
            w_t[i] = w_u

        def st_m2(i):
            vg = vg_t[i]
            w_u = w_t[i]
            nc.vector.tensor_tensor(
                out=vg[:].rearrange("p u (e h) -> p u e h", h=H),
                in0=vg[:].rearrange("p u (e h) -> p u e h", h=H),
                in1=w_u[:].rearrange("p r c h -> p (r c) h").unsqueeze(2)
                    .to_broadcast([128, UP, DH, H]),
                op=MULT)

        def st_c1(i):
            vg = vg_t[i]
            pv = vg[:].rearrange("p (r c) e -> p r c e", r=RP)
            nc.vector.tensor_tensor(out=pv[:, :, 0:4, :], in0=pv[:, :, 0:4, :],
                                    in1=pv[:, :, 4:8, :], op=ADD)

        def st_cd(i):
            t, u = units[i]
            vg = vg_t.pop(i)
            pv = vg[:].rearrange("p (r c) e -> p r c e", r=RP)
            nc.vector.tensor_tensor(out=pv[:, :, 0:2, :], in0=pv[:, :, 0:2, :],
                                    in1=pv[:, :, 2:4, :], op=ADD)
            nc.vector.tensor_tensor(out=pv[:, :, 0, :], in0=pv[:, :, 0, :],
                                    in1=pv[:, :, 1, :], op=ADD)
            nc.vector.tensor_tensor(out=pv[:, 0:2, 0, :], in0=pv[:, 0:2, 0, :],
                                    in1=pv[:, 2:4, 0, :], op=ADD)
            ctxu = cp.tile([128, D], F16, tag=f"ctxu{t}_{u}")
            nc.vector.tensor_tensor(out=ctxu[:], in0=pv[:, 0, 0, :],
                                    in1=pv[:, 1, 0, :], op=ADD)
            ctx_parts[t][u] = ctxu
            # z-tree on Pool (w_u stays intact for nothing else; reduce c)
            w_u = w_t.pop(i)
            nc.gpsimd.tensor_tensor(out=w_u[:, :, 0:4, :], in0=w_u[:, :, 0:4, :],
                                    in1=w_u[:, :, 4:8, :], op=ADD)
            nc.gpsimd.tensor_tensor(out=w_u[:, :, 0:2, :], in0=w_u[:, :, 0:2, :],
                                    in1=w_u[:, :, 2:4, :], op=ADD)
            zu = zp.tile([128, RP, H], F16, tag=f"zu{t}_{u}")
            nc.gpsimd.tensor_tensor(out=zu[:], in0=w_u[:, :, 0, :],
                                    in1=w_u[:, :, 1, :], op=ADD)
            z_parts[t][u] = zu

        # ---- phase O (emitted inside pipeline) ----
        def st_o(t):
                za = op_.tile([128, RP, H], F16, tag="za")
                nc.vector.tensor_tensor(out=za[:], in0=z_parts[t][0][:],
                                        in1=z_parts[t][1][:], op=ADD)
                nc.vector.tensor_tensor(out=za[:, 0:2, :], in0=za[:, 0:2, :],
                                        in1=za[:, 2:4, :], op=ADD)
                zs = op_.tile([128, H], F16, tag="zs")
                nc.vector.tensor_tensor(out=zs[:], in0=za[:, 0, :],
                                        in1=za[:, 1, :], op=ADD)
                zr = op_.tile([128, H], F16, tag="zr")
                nc.vector.reciprocal(out=zr[:], in_=zs[:])

                cs = op_.tile([128, D], F16, tag="cs")
                nc.vector.tensor_tensor(out=cs[:], in0=ctx_parts[t][0][:],
                                        in1=ctx_parts[t][1][:], op=ADD)
                ctxn = op_.tile([128, D], F32, tag="ctxn")
                nc.vector.tensor_tensor(
                    out=ctxn[:].rearrange("p (e h) -> p e h", h=H),
                    in0=cs[:].rearrange("p (e h) -> p e h", h=H),
                    in1=zr[:].unsqueeze(1).to_broadcast([128, DH, H]),
                    op=MULT)

                # transpose + out-proj
                ctxT = op_.tile([128, 2, 128], F16, tag="ctxT")
                for ch in range(2):
                    ptr = psC.tile([128, 128], F32, space="PSUM", tag="ptr")
                    nc.tensor.transpose(out=ptr[:], in_=ctxn[:, bass.ts(ch, 128)],
                                        identity=ident[:])
                    nc.scalar.copy(out=ctxT[:, ch, :], in_=ptr[:])
                po = psB.tile([128, D], F32, space="PSUM", tag="p256")
                nc.tensor.matmul(out=po[:], lhsT=ctxT[:, 0, :], rhs=owT_t[:, 0, :],
                                 start=True, stop=False)
                nc.tensor.matmul(out=po[:], lhsT=ctxT[:, 1, :], rhs=owT_t[:, 1, :],
                                 start=False, stop=True)
                o_sb = op_.tile([128, D], F32, tag="osb")
                nc.scalar.activation(out=o_sb[:], in_=po[:],
                                     func=mybir.ActivationFunctionType.Relu)
                nc.sync.dma_start(out=out[bass.ts(t, 128), :], in_=o_sb[:])


        schedule = [(st_gk, 0), (st_gv, 1), (st_m1, 1), (st_t1, 2),
                    (st_td, 2), (st_e, 2), (st_m2, 3), (st_c1, 3), (st_cd, 4)]
        MAXLAG = max(lag for _, lag in schedule)
        for i in range(NUNITS + MAXLAG):
            for fn_, lag in schedule:
                j = i - lag
                if 0 <= j < NUNITS:
                    fn_(j)
            if (i - 4) % NU == NU - 1 and 0 <= (i - 4) < NUNITS:
                st_o((i - 4) // NU)

    return nc


# ---------------------------------------------------------------------------
def host_prep(x, incidence, edge_attr, W_lin, W_edge,
              in_proj_w, in_proj_b, out_proj_w, out_proj_b):
    x = np.asarray(x, np.float32)
    inc = np.asarray(incidence, np.float32)
    ea = np.asarray(edge_attr, np.float32)
    W_lin = np.asarray(W_lin, np.float32)
    W_edge = np.asarray(W_edge, np.float32)
    in_proj_w = np.asarray(in_proj_w, np.float32)
    in_proj_b = np.asarray(in_proj_b, np.float32)
    out_proj_w = np.asarray(out_proj_w, np.float32)
    out_proj_b = np.asarray(out_proj_b, np.float32)
    assert not in_proj_b.any() and not out_proj_b.any(), "nonzero bias unsupported"

    # index lists from incidence (order within a node's pair set is irrelevant:
    # attention is permutation-invariant over the L pairs)
    eon = np.nonzero(inc.T)[1].reshape(N, DEG).astype(np.int32)   # edge_of_node
    noe = np.nonzero(inc)[1].reshape(E, CARD).astype(np.int32)    # node_of_edge
    pair_u = noe[eon].astype(np.int32)                            # [N, DEG, CARD]
    # move the self pair (u == n) to slot c=0 of each round
    nn = np.arange(N)[:, None]
    self_c = np.argmax(pair_u == nn[:, :, None][..., 0:1] * 0 + nn[:, None, :]
                       if False else (pair_u == np.arange(N)[:, None, None]), axis=2)
    for r in range(DEG):
        c0 = self_c[:, r]
        row = pair_u[np.arange(N), r, c0].copy()
        pair_u[np.arange(N), r, c0] = pair_u[:, r, 0]
        pair_u[:, r, 0] = row
    assert (pair_u[:, :, 0] == np.arange(N)[:, None]).all()
    pair_u = pair_u.reshape(N, L)
    pair_e = eon

    Wq, Wk, Wv = in_proj_w[0:D], in_proj_w[D:2 * D], in_proj_w[2 * D:3 * D]
    scale = 1.0 / np.sqrt(np.float32(DH))

    # (e,h)-interleaved column order: new col e*H+h <- old col h*DH+e
    perm = np.arange(D).reshape(H, DH).T.reshape(-1)

    wkc = (W_lin @ Wk.T)[:, perm]
    wvc = (W_lin @ Wv.T)[:, perm]
    wqc = (W_lin @ Wq.T * scale)[:, perm]
    wekc = (W_edge @ Wk.T)[:, perm]
    owT = out_proj_w.T[perm, :].copy()

    import ml_dtypes
    f16 = ml_dtypes.bfloat16
    rep = dict(
        xT=np.ascontiguousarray(x.T).astype(f16),
        eaT=np.ascontiguousarray(ea.T).astype(f16),
        wk=wkc.astype(f16), wv=wvc.astype(f16), wq=wqc.astype(f16),
        wek=wekc.astype(f16), owT=owT.astype(f16),
    )
    per_core = []
    for c in range(NCORES):
        sl = slice(c * NSH, (c + 1) * NSH)
        m = dict(rep)
        m["xT_own"] = np.ascontiguousarray(x.T[:, sl]).astype(f16)
        m["pu"] = pair_u[sl]
        m["pe"] = pair_e[sl]
        per_core.append(m)
    return per_core


_CACHE = {}


def kernel(x, incidence, edge_attr, W_lin, W_edge,
           in_proj_w, in_proj_b, out_proj_w, out_proj_b, deg, card):
    assert int(deg) == DEG and int(card) == CARD
    in_maps = host_prep(x, incidence, edge_attr, W_lin, W_edge,
                        in_proj_w, in_proj_b, out_proj_w, out_proj_b)
    if "nc" not in _CACHE:
        _CACHE["nc"] = build_nc()
    from concourse.bass_utils import run_bass_kernel_spmd
    res = run_bass_kernel_spmd(_CACHE["nc"], in_maps, list(range(NCORES)))
    return np.concatenate([res.results[c]["out"] for c in range(NCORES)], axis=0)

